# revision 12
# baseline (speedup 1.0000x reference)
"""GCNCheb Trainium2 kernel: out[b,n,fo] = sum_k T_k[b,n,:] @ W[k] + bias.

T_k recurrence (matrix powers P_j = L^j x with T0=P0, T1=P1, Tk=2*P_k - T_{k-2})
is linear, so the K/F_in contraction is re-expressed over pure powers with
host-precombined weights V_j:
    out = P0 (W0-W2) + P1 (W1-W3) + P2 (2 W2) + P3 (2 W3) + bias

Distribution over 8 NeuronCores: 1D row-shard of L. Core r holds the column
slice Lc_r = L[:, r*1024:(r+1)*1024] (== L[rows_r,:].T since L is symmetric),
bf16, fully SBUF-resident (16 MB), streamed in on the scalar HWDGE queue
(which is reserved for the L load: it is the critical path of the first
~45 us). X is [N, B*F_in] = [8192, 128] (batch folded into columns).

Steps 1-2 compute the core's 1024 rows of X_next = L @ X via per-m-tile PSUM
accumulations; step 3 computes P3^T = X2^T Lc directly (512-wide matmuls).

Collective schedule (the critical lever): exactly 4 AllGathers (half-shards
of X1 and X2). Mesh collectives execute serially on the CC firmware with
~7-10 us fixed cost each, and cannot make progress while the L load
saturates HBM/SDMA — so fewer, larger gathers win. Consuming steps issue
k-tiles in half-arrival order so each step starts on gather-half A while
half B is still in flight. Gather copy-backs use 1KB-contiguous descriptors
on the sync queue.

The projection contracts P_j^T tiles (built with PE transposes) with
block-diagonal weights packing all 4 batches, producing out^T per core; the
host untangles layout and adds bias.
"""

import os
import sys

sys.path.insert(0, "/opt/trn_rl_repo")

import numpy as np

import concourse.bass as bass
import concourse.mybir as mybir
import concourse.tile as tile
from concourse import bacc, bass_utils
from concourse.masks import make_identity

B, N, F_IN, F_OUT, K = 4, 8192, 32, 64, 4
NCORES = 8
P = 128
SH = N // NCORES          # rows per core (1024)
SH4 = SH // 4             # quarter-shard columns (256)
BF = B * F_IN             # folded X columns (128)
KT = N // P               # contraction tiles (64)
MT = SH // P              # output row tiles per core (8)
MH = MT // 2              # half-shard m-tiles (4)
QH = 2                    # output halves: (b in {2h, 2h+1}) x F_OUT = 128 partitions

_DT = {"bf16": mybir.dt.bfloat16, "fp32": mybir.dt.float32}


def _np_dt(variant):
    if variant == "bf16":
        import ml_dtypes

        return np.dtype(ml_dtypes.bfloat16)
    return np.dtype(np.float32)


def build_nc(variant="bf16"):
    dt = _DT[variant]
    f32 = mybir.dt.float32

    nc = bacc.Bacc()
    # all pre-tiled on host: partition-major, fully contiguous per partition
    Lc = nc.dram_tensor("Lc", [4, P, KT, SH4], dt, kind="ExternalInput")
    X0 = nc.dram_tensor("X0", [P, KT, BF], dt, kind="ExternalInput")
    X0T = nc.dram_tensor("X0T", [BF, SH], dt, kind="ExternalInput")
    WH = nc.dram_tensor("WH", [K, QH, BF, P], dt, kind="ExternalInput")
    OUT = nc.dram_tensor("OUT", [QH, P, SH], f32, kind="ExternalOutput")

    def kts_of(mt0, nmt):
        return [r * MT + mt0 + m for r in range(NCORES) for m in range(nmt)]

    ktH = [kts_of(0, MH), kts_of(MH, MH)]  # gather-half arrival order

    with tile.TileContext(nc) as tc:
        with (
            tc.tile_pool(name="lres", bufs=1) as lres_pool,
            tc.tile_pool(name="xbuf", bufs=2) as x_pool,
            tc.tile_pool(name="ybuf", bufs=2) as y_pool,
            tc.tile_pool(name="proj", bufs=1) as proj_pool,
            tc.tile_pool(name="psum", bufs=1, space="PSUM") as psum_pool,
            tc.tile_pool(name="dram", bufs=1, space="DRAM") as dram_pool,
        ):
            # --- initial loads: X first on sync (everything waits on it);
            # the scalar queue is dedicated to the resident L load ---
            x0 = x_pool.tile([P, KT, BF], dt, tag="x", name="x0")
            nc.sync.dma_start(x0[:, :2, :], X0[:, :2, :])
            nc.sync.dma_start(x0[:, 2:8, :], X0[:, 2:8, :])
            nc.sync.dma_start(x0[:, 8:32, :], X0[:, 8:32, :])
            nc.sync.dma_start(x0[:, 32:, :], X0[:, 32:, :])

            # L chunks: small leading pieces so the first matmuls start fast,
            # then 2MB chunks — per-DMA fixed cost (~2us completion receipt)
            # is what stretched a 33-chunk load to 62us.
            lc_res = lres_pool.tile([P, 4, KT, SH4], dt, tag="lc_res")
            for q in range(4):
                splits = [0, 2, 6, 32, 64] if q == 0 else [0, 32, 64]
                for a, b in zip(splits, splits[1:]):
                    nc.scalar.dma_start(
                        lc_res[:, q, a:b, :], Lc[q, :, a:b, :]
                    )



            whs = proj_pool.tile([P, K, QH, P], dt, tag="whs")
            nc.sync.dma_start(whs[:], WH.rearrange("k h p m -> p k h m"))
            pt = [proj_pool.tile([P, SH], dt, tag=f"pt{j}", name=f"pt{j}")
                  for j in range(K)]
            nc.sync.dma_start(pt[0][:], X0T[:, :])
            ident = proj_pool.tile([P, P], dt, tag="ident")
            make_identity(nc, ident[:])
            out_sb = proj_pool.tile([P, QH, 2, 512], f32, tag="out_sb")

            def lhsT_res(kt, mt):
                q, m = divmod(mt, 2)
                return lc_res[:, q, kt, m * P : (m + 1) * P]

            def finish_phase(yshd, ypsum, mts, ptj):
                """PSUM -> yshd (bf16), PE-transpose into P_j^T for projection."""
                for mt in mts:
                    nc.vector.tensor_copy(yshd[:, mt, :], ypsum[mt][:])
                for mt in mts:
                    tp = psum_pool.tile(
                        [P, P], dt, tag=f"ps{mt}", name=f"tp_{ptj.name}_{mt}"
                    )
                    nc.tensor.transpose(tp[:], yshd[:, mt, :], ident[:])
                    nc.vector.tensor_copy(ptj[:, mt * P : (mt + 1) * P], tp[:])

            def gather_half(step, h, yshd):
                """Shard-out DMA + AllGather for m-tile half h of this step."""
                mt0 = h * MH
                shard = dram_pool.tile(
                    [P, MH, BF], dt, tag=f"shard{step}_{h}",
                    name=f"shard{step}_{h}",
                )
                full = dram_pool.tile(
                    [NCORES * P, MH, BF], dt, addr_space="Shared",
                    tag=f"full{step}_{h}", name=f"full{step}_{h}",
                )
                nc.sync.dma_start(shard.opt(), yshd[:, mt0 : mt0 + MH, :])
                nc.gpsimd.collective_compute(
                    "AllGather",
                    mybir.AluOpType.bypass,
                    replica_groups=[list(range(NCORES))],
                    ins=[shard.opt()],
                    outs=[full.opt()],
                )
                return full

            def copy_back(full, x_nxt, h):
                """Gathered half -> SBUF x layout; 1KB-contiguous descriptors."""
                mt0 = h * MH
                xv = x_nxt[:].rearrange("p (r mt) f -> p r mt f", r=NCORES)
                fv = full[:].rearrange("(r p) mt f -> p r mt f", p=P)
                nc.sync.dma_start(xv[:, :, mt0 : mt0 + MH, :], fv[:])

            # ---------------- step 1: paced by L quarter arrival ----------------
            yshd1 = y_pool.tile([P, MT, BF], dt, tag="yshd", name="yshd1")
            x1 = x_pool.tile([P, KT, BF], dt, tag="x", name="x1")
            ypsum = {
                mt: psum_pool.tile([P, BF], f32, tag=f"ps{mt}", name=f"y1_{mt}")
                for mt in range(MT)
            }
            # tile_wait_until stamps are scheduler-sim ordering floors: the
            # monotone ladder pins the per-engine instruction order to the
            # intended pipeline (the scheduler otherwise reorders by its own
            # simulated readiness, and in-order engines then head-of-line
            # block: e.g. a gather copy-back scheduled ahead of the next
            # shard-out chains every mesh behind the previous one's
            # completion). Values are far above simulated times so ordering
            # is purely floor-driven; they emit no hardware waits.
            fulls1 = []
            for q in range(4):
                for kt in range(KT):
                    for mt in (2 * q, 2 * q + 1):
                        nc.tensor.matmul(
                            ypsum[mt][:],
                            lhsT=lhsT_res(kt, mt),
                            rhs=x0[:, kt, :],
                            start=(kt == 0),
                            stop=(kt == KT - 1),
                        )
                finish_phase(yshd1, ypsum, (2 * q, 2 * q + 1), pt[1])
                if q % 2 == 1:
                    with tc.tile_wait_until(0.30 + 0.02 * (q // 2)):
                        fulls1.append(gather_half(1, q // 2, yshd1))
            for h, full in enumerate(fulls1):
                with tc.tile_wait_until(0.35 + 0.04 * h):
                    copy_back(full, x1, h)

            # ---------------- step 2: consume X1 in half-arrival order ----------
            yshd2 = y_pool.tile([P, MT, BF], dt, tag="yshd", name="yshd2")
            x2 = x_pool.tile([P, KT, BF], dt, tag="x", name="x2")
            ypsum2 = {
                mt: psum_pool.tile([P, BF], f32, tag=f"ps{mt}", name=f"y2_{mt}")
                for mt in range(MT)
            }
            with tc.tile_wait_until(0.40):
                for hm in (0, 1):
                    for kt in ktH[0]:
                        for mt in range(hm * MH, hm * MH + MH):
                            nc.tensor.matmul(
                                ypsum2[mt][:],
                                lhsT=lhsT_res(kt, mt),
                                rhs=x1[:, kt, :],
                                start=(kt == ktH[0][0]),
                                stop=False,
                            )
            fulls2 = []
            for hm in (0, 1):
                with tc.tile_wait_until(0.42 + 0.04 * hm):
                    for kt in ktH[1]:
                        for mt in range(hm * MH, hm * MH + MH):
                            nc.tensor.matmul(
                                ypsum2[mt][:],
                                lhsT=lhsT_res(kt, mt),
                                rhs=x1[:, kt, :],
                                start=False,
                                stop=(kt == ktH[1][-1]),
                            )
                with tc.tile_wait_until(0.44 + 0.04 * hm):
                    finish_phase(
                        yshd2, ypsum2, range(hm * MH, hm * MH + MH), pt[2]
                    )
                    fulls2.append(gather_half(2, hm, yshd2))
            for h, full in enumerate(fulls2):
                with tc.tile_wait_until(0.49 + 0.04 * h):
                    copy_back(full, x2, h)

            # ------- step 3: P3^T = X2^T Lc (512-wide), projection interleaved ---
            pp3 = [
                psum_pool.tile([P, 512], f32, tag=f"ps{ns * 4}", name=f"p3_{ns}")
                for ns in (0, 1)
            ]
            with tc.tile_wait_until(0.54):
                for ns in (0, 1):
                    for kt in ktH[0]:
                        nc.tensor.matmul(
                            pp3[ns][:],
                            lhsT=x2[:, kt, :],
                            rhs=lc_res[:, 2 * ns : 2 * ns + 2, kt, :],
                            start=(kt == ktH[0][0]),
                            stop=False,
                        )
            for ns in (0, 1):
                with tc.tile_wait_until(0.55 + 0.02 * ns):
                    for kt in ktH[1]:
                        nc.tensor.matmul(
                            pp3[ns][:],
                            lhsT=x2[:, kt, :],
                            rhs=lc_res[:, 2 * ns : 2 * ns + 2, kt, :],
                            start=False,
                            stop=(kt == ktH[1][-1]),
                        )
                with tc.tile_wait_until(0.56 + 0.02 * ns):
                    nc.vector.tensor_copy(
                        pt[3][:, ns * 512 : (ns + 1) * 512], pp3[ns][:]
                    )
                    for h2 in range(QH):
                        pp = psum_pool.tile(
                            [P, 512], f32, tag=f"ps{ns * 4 + 1 + h2}",
                            name=f"pp{ns}_{h2}",
                        )
                        for j in range(K):
                            nc.tensor.matmul(
                                pp[:],
                                lhsT=whs[:, j, h2, :],
                                rhs=pt[j][:, ns * 512 : (ns + 1) * 512],
                                start=(j == 0),
                                stop=(j == K - 1),
                            )
                        nc.vector.tensor_copy(out_sb[:, h2, ns, :], pp[:])

            # final output, split across both HWDGE queues
            with tc.tile_wait_until(0.60):
                ov = OUT.rearrange("h q (s n) -> q h s n", s=2)
                nc.sync.dma_start(ov[:, 0, :, :], out_sb[:, 0, :, :])
                nc.scalar.dma_start(ov[:, 1, :, :], out_sb[:, 1, :, :])

    nc.compile()
    return nc


_CACHED = {}


def _get_nc(variant):
    if variant not in _CACHED:
        _CACHED[variant] = build_nc(variant)
    return _CACHED[variant]


def _prep_inputs(x, L, weight, variant):
    np_dt = _np_dt(variant)
    f32 = np.float32

    X0 = np.ascontiguousarray(
        x.astype(f32).transpose(1, 0, 2).reshape(N, BF)
    )  # [N, (b,fi)]
    X0_t = np.ascontiguousarray(
        X0.reshape(KT, P, BF).transpose(1, 0, 2)
    ).astype(np_dt)  # [P, KT, BF]
    W = weight.astype(f32)
    V = np.stack(
        [W[0] - W[2], W[1] - W[3], 2.0 * W[2], 2.0 * W[3]]
    )  # [4, F_IN, F_OUT]
    # block-diagonal packing: WH[j, h, b*F_IN+fi, bl*F_OUT+fo] = V[j,fi,fo]
    # for b == 2h + bl
    WH = np.zeros((K, QH, BF, P), dtype=f32)
    for j in range(K):
        for b in range(B):
            h, bl = divmod(b, 2)
            WH[j, h, b * F_IN : (b + 1) * F_IN, bl * F_OUT : (bl + 1) * F_OUT] = V[j]
    WH = WH.astype(np_dt)

    in_maps = []
    for r in range(NCORES):
        rows = slice(r * SH, (r + 1) * SH)
        Lc_r = np.ascontiguousarray(
            L[:, rows].reshape(KT, P, 4, SH4).transpose(2, 1, 0, 3)
        ).astype(np_dt)  # [4, P, KT, SH4]
        X0T_r = np.ascontiguousarray(X0[rows, :].T).astype(np_dt)
        in_maps.append({"Lc": Lc_r, "X0": X0_t, "X0T": X0T_r, "WH": WH})
    return in_maps


def _assemble(results, bias):
    out = np.empty((B, N, F_OUT), dtype=np.float32)
    for r in range(NCORES):
        outT = results[r]["OUT"]  # [QH, 128, SH]
        for b in range(B):
            h, bl = divmod(b, 2)
            out[b, r * SH : (r + 1) * SH, :] = outT[
                h, bl * F_OUT : (bl + 1) * F_OUT, :
            ].T
    out += bias.astype(np.float32)
    return out


def run(x, L, weight, bias, variant="bf16", trace=False):
    nc = _get_nc(variant)
    in_maps = _prep_inputs(x, L, weight, variant)
    last_err = None
    for attempt in range(4):
        try:
            res = bass_utils.run_bass_kernel_spmd(
                nc,
                in_maps,
                core_ids=list(range(NCORES)),
                trace=trace,
                trace_cores=list(range(NCORES)) if trace else None,
            )
            out = _assemble(res.results, bias)
            if np.isnan(out).any() or np.isinf(out).any():
                # transient device wedge can return garbage: retry
                raise RuntimeError("non-finite output from device")
            break
        except Exception as e:
            last_err = e
            import time

            time.sleep(5)
    else:
        raise last_err
    return out, res


def kernel(x, L, weight, bias):
    out, _ = run(
        np.asarray(x), np.asarray(L), np.asarray(weight), np.asarray(bias)
    )
    return out


# revision 13
# speedup vs baseline: 1.0587x; 1.0587x over previous
"""GCNCheb Trainium2 kernel: out[b,n,fo] = sum_k T_k[b,n,:] @ W[k] + bias.

T_k recurrence (matrix powers P_j = L^j x with T0=P0, T1=P1, Tk=2*P_k - T_{k-2})
is linear, so the K/F_in contraction is re-expressed over pure powers with
host-precombined weights V_j:
    out = P0 (W0-W2) + P1 (W1-W3) + P2 (2 W2) + P3 (2 W3) + bias

Distribution over 8 NeuronCores: 1D row-shard of L. Core r holds the column
slice Lc_r = L[:, r*1024:(r+1)*1024] (== L[rows_r,:].T since L is symmetric),
bf16, fully SBUF-resident (16 MB), streamed in on the scalar HWDGE queue
(which is reserved for the L load: it is the critical path of the first
~45 us). X is [N, B*F_in] = [8192, 128] (batch folded into columns).

Steps 1-2 compute the core's 1024 rows of X_next = L @ X via per-m-tile PSUM
accumulations; step 3 computes P3^T = X2^T Lc directly (512-wide matmuls).

Collective schedule (the critical lever): exactly 4 AllGathers (half-shards
of X1 and X2). Mesh collectives execute serially on the CC firmware with
~7-10 us fixed cost each, and cannot make progress while the L load
saturates HBM/SDMA — so fewer, larger gathers win. Consuming steps issue
k-tiles in half-arrival order so each step starts on gather-half A while
half B is still in flight. Gather copy-backs use 1KB-contiguous descriptors
on the sync queue.

The projection contracts P_j^T tiles (built with PE transposes) with
block-diagonal weights packing all 4 batches, producing out^T per core; the
host untangles layout and adds bias.
"""

import os
import sys

sys.path.insert(0, "/opt/trn_rl_repo")

import numpy as np

import concourse.bass as bass
import concourse.mybir as mybir
import concourse.tile as tile
from concourse import bacc, bass_utils
from concourse.masks import make_identity

B, N, F_IN, F_OUT, K = 4, 8192, 32, 64, 4
NCORES = 8
P = 128
SH = N // NCORES          # rows per core (1024)
SH4 = SH // 4             # quarter-shard columns (256)
BF = B * F_IN             # folded X columns (128)
KT = N // P               # contraction tiles (64)
MT = SH // P              # output row tiles per core (8)
MH = MT // 2              # half-shard m-tiles (4)
QH = 2                    # output halves: (b in {2h, 2h+1}) x F_OUT = 128 partitions

_DT = {"bf16": mybir.dt.bfloat16, "fp32": mybir.dt.float32}


def _np_dt(variant):
    if variant == "bf16":
        import ml_dtypes

        return np.dtype(ml_dtypes.bfloat16)
    return np.dtype(np.float32)


def build_nc(variant="bf16"):
    dt = _DT[variant]
    f32 = mybir.dt.float32

    nc = bacc.Bacc()
    # all pre-tiled on host: partition-major, fully contiguous per partition
    Lc = nc.dram_tensor("Lc", [4, P, KT, SH4], dt, kind="ExternalInput")
    X0 = nc.dram_tensor("X0", [P, KT, BF], dt, kind="ExternalInput")
    X0T = nc.dram_tensor("X0T", [BF, SH], dt, kind="ExternalInput")
    WH = nc.dram_tensor("WH", [K, QH, BF, P], dt, kind="ExternalInput")
    OUT = nc.dram_tensor("OUT", [QH, P, SH], dt, kind="ExternalOutput")

    def kts_of(mt0, nmt):
        return [r * MT + mt0 + m for r in range(NCORES) for m in range(nmt)]

    ktH = [kts_of(0, MH), kts_of(MH, MH)]  # gather-half arrival order

    with tile.TileContext(nc) as tc:
        with (
            tc.tile_pool(name="lres", bufs=1) as lres_pool,
            tc.tile_pool(name="xbuf", bufs=2) as x_pool,
            tc.tile_pool(name="ybuf", bufs=2) as y_pool,
            tc.tile_pool(name="proj", bufs=1) as proj_pool,
            tc.tile_pool(name="psum", bufs=1, space="PSUM") as psum_pool,
            tc.tile_pool(name="dram", bufs=1, space="DRAM") as dram_pool,
        ):
            # --- initial loads: X first on sync (everything waits on it);
            # the scalar queue is dedicated to the resident L load ---
            x0 = x_pool.tile([P, KT, BF], dt, tag="x", name="x0")
            nc.sync.dma_start(x0[:, :2, :], X0[:, :2, :])
            nc.sync.dma_start(x0[:, 2:8, :], X0[:, 2:8, :])
            nc.sync.dma_start(x0[:, 8:32, :], X0[:, 8:32, :])
            nc.sync.dma_start(x0[:, 32:, :], X0[:, 32:, :])

            # L chunks: small leading pieces so the first matmuls start fast,
            # then 2MB chunks — per-DMA fixed cost (~2us completion receipt)
            # is what stretched a 33-chunk load to 62us.
            lc_res = lres_pool.tile([P, 4, KT, SH4], dt, tag="lc_res")
            for q in range(4):
                splits = [0, 2, 6, 32, 64] if q == 0 else [0, 32, 64]
                for a, b in zip(splits, splits[1:]):
                    nc.scalar.dma_start(
                        lc_res[:, q, a:b, :], Lc[q, :, a:b, :]
                    )



            whs = proj_pool.tile([P, K, QH, P], dt, tag="whs")
            nc.sync.dma_start(whs[:], WH.rearrange("k h p m -> p k h m"))
            pt = [proj_pool.tile([P, SH], dt, tag=f"pt{j}", name=f"pt{j}")
                  for j in range(K)]
            nc.sync.dma_start(pt[0][:], X0T[:, :])
            ident = proj_pool.tile([P, P], dt, tag="ident")
            make_identity(nc, ident[:])
            out_sb = proj_pool.tile([P, QH, 2, 512], dt, tag="out_sb")

            def lhsT_res(kt, mt):
                q, m = divmod(mt, 2)
                return lc_res[:, q, kt, m * P : (m + 1) * P]

            def finish_phase(yshd, ypsum, mts, ptj):
                """PSUM -> yshd (bf16), PE-transpose into P_j^T for projection."""
                for mt in mts:
                    nc.vector.tensor_copy(yshd[:, mt, :], ypsum[mt][:])
                for mt in mts:
                    tp = psum_pool.tile(
                        [P, P], dt, tag=f"ps{mt}", name=f"tp_{ptj.name}_{mt}"
                    )
                    nc.tensor.transpose(tp[:], yshd[:, mt, :], ident[:])
                    nc.vector.tensor_copy(ptj[:, mt * P : (mt + 1) * P], tp[:])

            def gather_half(step, h, yshd):
                """Shard-out DMA + AllGather for m-tile half h of this step."""
                mt0 = h * MH
                shard = dram_pool.tile(
                    [P, MH, BF], dt, tag=f"shard{step}_{h}",
                    name=f"shard{step}_{h}",
                )
                full = dram_pool.tile(
                    [NCORES * P, MH, BF], dt, addr_space="Shared",
                    tag=f"full{step}_{h}", name=f"full{step}_{h}",
                )
                nc.sync.dma_start(shard.opt(), yshd[:, mt0 : mt0 + MH, :])
                nc.gpsimd.collective_compute(
                    "AllGather",
                    mybir.AluOpType.bypass,
                    replica_groups=[list(range(NCORES))],
                    ins=[shard.opt()],
                    outs=[full.opt()],
                )
                return full

            def copy_back(full, x_nxt, h):
                """Gathered half -> SBUF x layout; 1KB-contiguous descriptors."""
                mt0 = h * MH
                xv = x_nxt[:].rearrange("p (r mt) f -> p r mt f", r=NCORES)
                fv = full[:].rearrange("(r p) mt f -> p r mt f", p=P)
                nc.sync.dma_start(xv[:, :, mt0 : mt0 + MH, :], fv[:])

            # ---------------- step 1: paced by L quarter arrival ----------------
            yshd1 = y_pool.tile([P, MT, BF], dt, tag="yshd", name="yshd1")
            x1 = x_pool.tile([P, KT, BF], dt, tag="x", name="x1")
            ypsum = {
                mt: psum_pool.tile([P, BF], f32, tag=f"ps{mt}", name=f"y1_{mt}")
                for mt in range(MT)
            }
            # tile_wait_until stamps are scheduler-sim ordering floors: the
            # monotone ladder pins the per-engine instruction order to the
            # intended pipeline (the scheduler otherwise reorders by its own
            # simulated readiness, and in-order engines then head-of-line
            # block: e.g. a gather copy-back scheduled ahead of the next
            # shard-out chains every mesh behind the previous one's
            # completion). Values are far above simulated times so ordering
            # is purely floor-driven; they emit no hardware waits.
            fulls1 = []
            for q in range(4):
                with tc.tile_wait_until(0.10 + 0.02 * q):
                    for kt in range(KT):
                        for mt in (2 * q, 2 * q + 1):
                            nc.tensor.matmul(
                                ypsum[mt][:],
                                lhsT=lhsT_res(kt, mt),
                                rhs=x0[:, kt, :],
                                start=(kt == 0),
                                stop=(kt == KT - 1),
                            )
                with tc.tile_wait_until(0.105 + 0.02 * q):
                    finish_phase(yshd1, ypsum, (2 * q, 2 * q + 1), pt[1])
                if q % 2 == 1:
                    with tc.tile_wait_until(0.13 + 0.04 * (q // 2)):
                        fulls1.append(gather_half(1, q // 2, yshd1))
            for h, full in enumerate(fulls1):
                with tc.tile_wait_until(0.18 + 0.035 * h):
                    copy_back(full, x1, h)

            # ---------------- step 2: consume X1 in half-arrival order ----------
            yshd2 = y_pool.tile([P, MT, BF], dt, tag="yshd", name="yshd2")
            x2 = x_pool.tile([P, KT, BF], dt, tag="x", name="x2")
            ypsum2 = {
                mt: psum_pool.tile([P, BF], f32, tag=f"ps{mt}", name=f"y2_{mt}")
                for mt in range(MT)
            }
            with tc.tile_wait_until(0.22):
                for hm in (0, 1):
                    for kt in ktH[0]:
                        for mt in range(hm * MH, hm * MH + MH):
                            nc.tensor.matmul(
                                ypsum2[mt][:],
                                lhsT=lhsT_res(kt, mt),
                                rhs=x1[:, kt, :],
                                start=(kt == ktH[0][0]),
                                stop=False,
                            )
            fulls2 = []
            for hm in (0, 1):
                with tc.tile_wait_until(0.24 + 0.04 * hm):
                    for kt in ktH[1]:
                        for mt in range(hm * MH, hm * MH + MH):
                            nc.tensor.matmul(
                                ypsum2[mt][:],
                                lhsT=lhsT_res(kt, mt),
                                rhs=x1[:, kt, :],
                                start=False,
                                stop=(kt == ktH[1][-1]),
                            )
                with tc.tile_wait_until(0.26 + 0.04 * hm):
                    finish_phase(
                        yshd2, ypsum2, range(hm * MH, hm * MH + MH), pt[2]
                    )
                    fulls2.append(gather_half(2, hm, yshd2))
            for h, full in enumerate(fulls2):
                with tc.tile_wait_until(0.31 + 0.035 * h):
                    copy_back(full, x2, h)

            # ------- step 3: P3^T = X2^T Lc (512-wide), projection interleaved ---
            pp3 = [
                psum_pool.tile([P, 512], f32, tag=f"ps{ns * 4}", name=f"p3_{ns}")
                for ns in (0, 1)
            ]
            with tc.tile_wait_until(0.35):
                for ns in (0, 1):
                    for kt in ktH[0]:
                        nc.tensor.matmul(
                            pp3[ns][:],
                            lhsT=x2[:, kt, :],
                            rhs=lc_res[:, 2 * ns : 2 * ns + 2, kt, :],
                            start=(kt == ktH[0][0]),
                            stop=False,
                        )
            for ns in (0, 1):
                with tc.tile_wait_until(0.36 + 0.02 * ns):
                    for kt in ktH[1]:
                        nc.tensor.matmul(
                            pp3[ns][:],
                            lhsT=x2[:, kt, :],
                            rhs=lc_res[:, 2 * ns : 2 * ns + 2, kt, :],
                            start=False,
                            stop=(kt == ktH[1][-1]),
                        )
                with tc.tile_wait_until(0.37 + 0.02 * ns):
                    nc.vector.tensor_copy(
                        pt[3][:, ns * 512 : (ns + 1) * 512], pp3[ns][:]
                    )
                    for h2 in range(QH):
                        pp = psum_pool.tile(
                            [P, 512], f32, tag=f"ps{ns * 4 + 1 + h2}",
                            name=f"pp{ns}_{h2}",
                        )
                        for j in range(K):
                            nc.tensor.matmul(
                                pp[:],
                                lhsT=whs[:, j, h2, :],
                                rhs=pt[j][:, ns * 512 : (ns + 1) * 512],
                                start=(j == 0),
                                stop=(j == K - 1),
                            )
                        nc.vector.tensor_copy(out_sb[:, h2, ns, :], pp[:])

            # final output, split across both HWDGE queues
            with tc.tile_wait_until(0.40):
                ov = OUT.rearrange("h q (s n) -> q h s n", s=2)
                nc.sync.dma_start(ov[:, 0, :, :], out_sb[:, 0, :, :])
                nc.scalar.dma_start(ov[:, 1, :, :], out_sb[:, 1, :, :])

    nc.compile()
    return nc


_CACHED = {}


def _get_nc(variant):
    if variant not in _CACHED:
        _CACHED[variant] = build_nc(variant)
    return _CACHED[variant]


def _prep_inputs(x, L, weight, variant):
    np_dt = _np_dt(variant)
    f32 = np.float32

    X0 = np.ascontiguousarray(
        x.astype(f32).transpose(1, 0, 2).reshape(N, BF)
    )  # [N, (b,fi)]
    X0_t = np.ascontiguousarray(
        X0.reshape(KT, P, BF).transpose(1, 0, 2)
    ).astype(np_dt)  # [P, KT, BF]
    W = weight.astype(f32)
    V = np.stack(
        [W[0] - W[2], W[1] - W[3], 2.0 * W[2], 2.0 * W[3]]
    )  # [4, F_IN, F_OUT]
    # block-diagonal packing: WH[j, h, b*F_IN+fi, bl*F_OUT+fo] = V[j,fi,fo]
    # for b == 2h + bl
    WH = np.zeros((K, QH, BF, P), dtype=f32)
    for j in range(K):
        for b in range(B):
            h, bl = divmod(b, 2)
            WH[j, h, b * F_IN : (b + 1) * F_IN, bl * F_OUT : (bl + 1) * F_OUT] = V[j]
    WH = WH.astype(np_dt)

    in_maps = []
    for r in range(NCORES):
        rows = slice(r * SH, (r + 1) * SH)
        Lc_r = np.ascontiguousarray(
            L[:, rows].reshape(KT, P, 4, SH4).transpose(2, 1, 0, 3)
        ).astype(np_dt)  # [4, P, KT, SH4]
        X0T_r = np.ascontiguousarray(X0[rows, :].T).astype(np_dt)
        in_maps.append({"Lc": Lc_r, "X0": X0_t, "X0T": X0T_r, "WH": WH})
    return in_maps


def _assemble(results, bias):
    out = np.empty((B, N, F_OUT), dtype=np.float32)
    for r in range(NCORES):
        outT = results[r]["OUT"].astype(np.float32)  # [QH, 128, SH]
        for b in range(B):
            h, bl = divmod(b, 2)
            out[b, r * SH : (r + 1) * SH, :] = outT[
                h, bl * F_OUT : (bl + 1) * F_OUT, :
            ].T
    out += bias.astype(np.float32)
    return out


def run(x, L, weight, bias, variant="bf16", trace=False):
    nc = _get_nc(variant)
    in_maps = _prep_inputs(x, L, weight, variant)
    last_err = None
    for attempt in range(4):
        try:
            res = bass_utils.run_bass_kernel_spmd(
                nc,
                in_maps,
                core_ids=list(range(NCORES)),
                trace=trace,
                trace_cores=list(range(NCORES)) if trace else None,
            )
            out = _assemble(res.results, bias)
            if np.isnan(out).any() or np.isinf(out).any():
                # transient device wedge can return garbage: retry
                raise RuntimeError("non-finite output from device")
            break
        except Exception as e:
            last_err = e
            import time

            time.sleep(5)
    else:
        raise last_err
    return out, res


def kernel(x, L, weight, bias):
    out, _ = run(
        np.asarray(x), np.asarray(L), np.asarray(weight), np.asarray(bias)
    )
    return out
